# revision 1
# baseline (speedup 1.0000x reference)
# Causal self-attention (B=2, T=4096, C=768, H=12, D=64) on 8 trn2 cores.
#
# Sharding: core c = 4*b + hg handles batch b and head-group hg (3 heads).
# Per core:
#   qT,kT (d-major) and v (t-major) via QKV matmuls against DMA-transposed x
#   flash-style causal attention in scoresT orientation (tk partitions, tq free)
#   softmax sums ride along as a ones-column appended to v (row 64 of yT)
#   per-head projection with w_proj rows, normalized by 1/sums on DVE while
#   accumulating heads; host sums the 4 head-group partials per batch.
import numpy as np

B, T, C = 2, 4096, 768
H, D = 12, 64
NHL = 3          # heads per core
CT = C // 128    # 6 contraction tiles

_PROG_CACHE = {}

# test-harness hooks (harmless when unused): set TRACE=True before calling
# kernel() to capture an NTFF profile; the BassKernelResults lands in LAST.
TRACE = False
LAST = None


def _build(t_len, has_ba):
    import concourse.bass as bass
    import concourse.bacc as bacc
    import concourse.mybir as mybir
    import concourse.tile as tile
    from concourse.bass import ts, ds

    f32 = mybir.dt.float32
    bf16 = mybir.dt.bfloat16
    AF = mybir.ActivationFunctionType
    ALU = mybir.AluOpType

    TB = t_len // 128   # 128-row t tiles
    QB = t_len // 512   # 512-row q blocks

    nc = bacc.Bacc("TRN2", target_bir_lowering=False, debug=False)

    x_d = nc.dram_tensor("x", [t_len, C], f32, kind="ExternalInput").ap()
    wqk_d = nc.dram_tensor("wqk", [C, 384], f32, kind="ExternalInput").ap()
    wv_d = nc.dram_tensor("wv", [C, NHL * D], f32, kind="ExternalInput").ap()
    wp_d = nc.dram_tensor("wp", [NHL * D, C], f32, kind="ExternalInput").ap()
    masks_d = nc.dram_tensor("masks", [128, 2048], bf16, kind="ExternalInput").ap()
    maskshh_d = nc.dram_tensor(
        "maskshh", [128, 4096], bf16, kind="ExternalInput"
    ).ap()
    if has_ba:
        baqk_d = nc.dram_tensor("baqk", [1, 384], f32, kind="ExternalInput").ap()
        bav_d = nc.dram_tensor("bav", [1, NHL * D], f32, kind="ExternalInput").ap()
    out_d = nc.dram_tensor("out", [t_len, C], f32, kind="ExternalOutput").ap()

    with tile.TileContext(nc) as tc:
        with (
            tc.tile_pool(name="const", bufs=1) as constp,
            tc.tile_pool(name="big", bufs=1) as bigp,
            tc.tile_pool(name="xload", bufs=1) as xloadp,
            tc.tile_pool(name="xTp", bufs=2) as xTp,
            tc.tile_pool(name="expp", bufs=8) as expp,
            tc.tile_pool(name="ytp", bufs=8) as ytp,
            tc.tile_pool(name="accp", bufs=3) as accp,
            tc.tile_pool(name="small", bufs=8) as smallp,
            tc.tile_pool(name="psA", bufs=2, space="PSUM") as psA,
            tc.tile_pool(name="psY", bufs=3, space="PSUM") as psY,
            tc.tile_pool(name="psP", bufs=1, space="PSUM") as psP,
        ):
            # ---- persistent SBUF tensors ----
            qkT = bigp.tile([128, 4, t_len], bf16)        # [q0|q1],[k0|k1],[q2|k2],[k2|-]
            vaug = bigp.tile([128, TB, NHL * 65], bf16)   # v + ones col per head
            wqk_sb = bigp.tile([128, CT, 384], bf16)
            wv_sb = bigp.tile([128, CT, NHL * D], bf16)
            wp_sb = bigp.tile([64, NHL, C], bf16)
            masks_sb = bigp.tile([128, 2048], bf16)
            maskshh_sb = bigp.tile([128, 4096], bf16)
            ones2 = constp.tile([33, 64], bf16)
            nc.vector.memset(ones2, 0.0)
            nc.vector.memset(ones2[0:1, :], 1.0)
            nc.vector.memset(ones2[32:33, :], 1.0)
            hilo_l = []
            for _h in range(NHL):
                hl = constp.tile([33, 512], bf16, name=f"hilo{_h}")
                nc.vector.memset(hl, 0.0)
                hilo_l.append(hl)
            if has_ba:
                baqk_sb = constp.tile([1, 384], bf16)
                bav_sb = constp.tile([1, NHL * D], bf16)
                ones128 = constp.tile([1, 128], bf16)
                ones512 = constp.tile([1, 512], bf16)
                nc.vector.memset(ones128, 1.0)
                nc.vector.memset(ones512, 1.0)
                nc.gpsimd.dma_start(out=baqk_sb, in_=baqk_d)
                nc.gpsimd.dma_start(out=bav_sb, in_=bav_d)

            nc.gpsimd.dma_start(
                out=wqk_sb, in_=wqk_d.rearrange("(ct p) n -> p ct n", p=128)
            )
            nc.gpsimd.dma_start(
                out=wv_sb, in_=wv_d.rearrange("(ct p) n -> p ct n", p=128)
            )
            nc.gpsimd.dma_start(
                out=wp_sb, in_=wp_d.rearrange("(h p) n -> p h n", p=64)
            )

            # ones columns of vaug (col 64 of each head's 65-wide chunk)
            vaug4 = vaug.rearrange("p t (h e) -> p t h e", e=65)
            for h in range(NHL):
                nc.vector.memset(vaug4[:, :, h, 64:65], 1.0)

            # scores operand slices: head -> (partition offset, qk tile indices)
            def q_ap(h, J):
                p0, nt = [(0, 0), (64, 0), (0, 2)][h]
                return qkT[p0 : p0 + 64, nt, ts(J, 512)]

            def k_ap(h, kt):
                p0, nt = [(0, 1), (64, 1), (0, 3)][h]
                return qkT[p0 : p0 + 64, nt, ts(kt, 128)]

            def qkv_parts(J):
                """Emit block J's x load + transposes now; return 8 closures
                (4 qk + 4 v matmul groups) to be interleaved into other loops."""
                xTb = xTp.tile([128, CT, 512], bf16, tag="xT")
                xbf = xloadp.tile([128, 4, C], bf16, tag="xbf")
                nc.gpsimd.dma_start(
                    out=xbf,
                    in_=x_d[ts(J, 512), :].rearrange("(a p) c -> p a c", p=128),
                )
                for sub in range(4):
                    nc.sync.dma_start(
                        out=xTb[:, :, ts(sub, 128)],
                        in_=xbf[:, sub, :],
                        transpose=True,
                    )

                def qk_group(nt):
                    qk_ps = psY.tile([128, 512], f32, tag="yt")
                    for ct in range(CT):
                        nc.tensor.matmul(
                            qk_ps,
                            wqk_sb[:, ct, ts(nt, 128)],
                            xTb[:, ct, :],
                            start=(ct == 0),
                            stop=(ct == CT - 1 and not has_ba),
                        )
                    if has_ba:
                        nc.tensor.matmul(
                            qk_ps, baqk_sb[:, ts(nt, 128)], ones512,
                            start=False, stop=True,
                        )
                    nc.vector.tensor_copy(out=qkT[:, nt, ts(J, 512)], in_=qk_ps)

                def v_group(sub):
                    tb = J * 4 + sub
                    v_ps = psY.tile([128, NHL * D], f32, tag="yt")
                    for ct in range(CT):
                        nc.tensor.matmul(
                            v_ps,
                            xTb[:, ct, ts(sub, 128)],
                            wv_sb[:, ct, :],
                            start=(ct == 0),
                            stop=(ct == CT - 1 and not has_ba),
                        )
                    if has_ba:
                        nc.tensor.matmul(
                            v_ps, ones128, bav_sb, start=False, stop=True
                        )
                    nc.vector.tensor_copy(
                        out=vaug4[:, tb, :, 0:64],
                        in_=v_ps.rearrange("p (h e) -> p h e", e=64),
                    )

                def k2_shift():
                    # nt3[0:64] = k2 (from nt2's upper half) so h2's scores
                    # get base-aligned operands without a 4th matmul group
                    nc.gpsimd.dma_start(
                        out=qkT[0:64, 3, ts(J, 512)],
                        in_=qkT[64:128, 2, ts(J, 512)],
                    )

                # qk tiles 0,1 first (h0/h1 scores depend on them), then v
                # (PV), then qk 2 + the k2 partition-shift (h2 comes last)
                return (
                    [lambda nt=nt: qk_group(nt) for nt in (0, 1)]
                    + [lambda sub=sub: v_group(sub) for sub in range(4)]
                    + [lambda: qk_group(2), k2_shift]
                )

            def qkv_block(J):
                for part in qkv_parts(J):
                    part()

            # normalize 1/sums into yts during evac: broadcast sums row to 64
            # partitions via a K=1 matmul, approx-reciprocal, multiply.
            def finalize(yT_ps, yts_l, h):
                # broadcast sums row to 64 partitions exactly: bf16 hi at
                # partition 0, residual lo at partition 32 (legal DVE base),
                # zeros between; one K=33 matmul against ones at rows {0,32}
                hilo = hilo_l[h]
                nc.vector.tensor_copy(out=hilo[0:1, :], in_=yT_ps[64:65, :])
                nc.vector.tensor_sub(hilo[32:33, :], yT_ps[64:65, :], hilo[0:1, :])
                bc_ps = psP.tile([64, 512], f32, tag="pj")
                nc.tensor.matmul(bc_ps, ones2, hilo, start=True, stop=True)
                rbc = smallp.tile([64, 512], f32, tag="rbc")
                nc.vector.reciprocal_approx_fast(rbc, bc_ps)
                yts = ytp.tile([64, 512], bf16, tag="yts")
                nc.vector.tensor_tensor(
                    out=yts, in0=yT_ps[0:64, :], in1=rbc, op=ALU.mult
                )
                yts_l[h] = yts

            def proj_parts(J, yts_l):
                """4 closures, one 128-row output tile each. The last block's
                projection uses the (by then idle) scores pool so its matmuls
                and evac copies double-buffer instead of ping-ponging."""
                pool, tag = (psA, "psa") if J == QB - 1 else (psP, "pj")

                def jj_part(jj):
                    pj = pool.tile([128, 512], f32, tag=tag)
                    for h in range(NHL):
                        nc.tensor.matmul(
                            pj,
                            yts_l[h][:, ts(jj, 128)],
                            wp_sb[:, h, 0:512],
                            start=(h == 0),
                            stop=(h == NHL - 1),
                        )
                    acc = accp.tile([128, C], f32, tag="acc")
                    nc.vector.tensor_copy(out=acc[:, 0:512], in_=pj)
                    pj2 = pool.tile([128, 256], f32, tag=tag)
                    for h in range(NHL):
                        nc.tensor.matmul(
                            pj2,
                            yts_l[h][:, ts(jj, 128)],
                            wp_sb[:, h, 512:768],
                            start=(h == 0),
                            stop=(h == NHL - 1),
                        )
                    nc.vector.tensor_copy(out=acc[:, 512:768], in_=pj2)
                    nc.sync.dma_start(
                        out=out_d[ds(J * 512 + jj * 128, 128), :], in_=acc
                    )

                return [lambda jj=jj: jj_part(jj) for jj in range(4)]

            # ---- pipelined: QKV(J+1) emitted inside h2(J)'s k-loop,
            # proj(J-1) inside h01(J)'s k-loop ----
            qkv_block(0)
            # mask loads emitted after block-0 QKV so the serial DMA engines
            # prioritize x/wqk (masks are first read ~20us in, by J=0's exps)
            nc.sync.dma_start(out=masks_sb, in_=masks_d)
            nc.sync.dma_start(out=maskshh_sb, in_=maskshh_d)
            pending_proj = []
            # heads 0+1 jointly: score tiles [h0 | h1] per single ktile;
            # the two score matmuls sit on row groups 0-63 / 64-127.
            # Software-pipelined: scores(k+1) are emitted before PV(k) so PE
            # stays a step ahead of the serial ACT exp chain.
            def koff(Jq, kt):
                # first valid q column for (possibly diagonal) ktile kt
                return 128 * (kt - 4 * Jq) if kt >= 4 * Jq else 0

            def scores_exp_h01(Jq, kt):
                    J = Jq
                    off = koff(Jq, kt)
                    sc = psA.tile([128, 1024], f32, tag="psa")
                    nc.tensor.matmul(
                        sc[:, off:512], k_ap(0, kt), q_ap(0, J)[:, off:512],
                        start=True, stop=True,
                    )
                    nc.tensor.matmul(
                        sc[:, 512 + off : 1024], k_ap(1, kt),
                        q_ap(1, J)[:, off:512],
                        start=True, stop=True,
                    )
                    ex = expp.tile([128, 1024], bf16, tag="ex")
                    sc2 = sc.rearrange("p (two n) -> p two n", two=2)
                    ex2 = ex.rearrange("p (two n) -> p two n", two=2)
                    nc.scalar.activation(
                        ex2[:, :, off:512], sc2[:, :, off:512], AF.Exp, scale=0.125
                    )
                    if kt >= 4 * J:
                        m4 = maskshh_sb.rearrange(
                            "p (j two n) -> p j two n", j=4, two=2
                        )
                        nc.vector.tensor_mul(
                            ex2[:, :, off:512],
                            ex2[:, :, off:512],
                            m4[:, kt - 4 * J, :, off:512],
                        )
                    return ex

            def scores_exp_h2(Jq, g):
                    J = Jq
                    offa, offb = koff(Jq, 2 * g), koff(Jq, 2 * g + 1)
                    sc = psA.tile([128, 1024], f32, tag="psa")
                    nc.tensor.matmul(
                        sc[:, offa:512], k_ap(2, 2 * g), q_ap(2, J)[:, offa:512],
                        start=True, stop=True,
                    )
                    nc.tensor.matmul(
                        sc[:, 512 + offb : 1024], k_ap(2, 2 * g + 1),
                        q_ap(2, J)[:, offb:512],
                        start=True, stop=True,
                    )
                    ex = expp.tile([128, 1024], bf16, tag="ex")
                    if offa == offb:
                        sc2 = sc.rearrange("p (two n) -> p two n", two=2)
                        ex2 = ex.rearrange("p (two n) -> p two n", two=2)
                        nc.scalar.activation(
                            ex2[:, :, offa:512], sc2[:, :, offa:512],
                            AF.Exp, scale=0.125,
                        )
                    else:
                        nc.scalar.activation(
                            ex[:, offa:512], sc[:, offa:512], AF.Exp, scale=0.125
                        )
                        nc.scalar.activation(
                            ex[:, 512 + offb : 1024], sc[:, 512 + offb : 1024],
                            AF.Exp, scale=0.125,
                        )
                    if g >= 2 * Jq:
                        ja, jb = 2 * g - 4 * Jq, 2 * g + 1 - 4 * Jq
                        m4 = masks_sb.rearrange("p (j n) -> p j n", j=4)
                        nc.vector.tensor_mul(
                            ex[:, offa:512],
                            ex[:, offa:512],
                            m4[:, ja, offa:512],
                        )
                        nc.vector.tensor_mul(
                            ex[:, 512 + offb : 1024],
                            ex[:, 512 + offb : 1024],
                            m4[:, jb, offb:512],
                        )
                    return ex

            for J in range(QB):
                nkt = 4 * (J + 1)
                yts_l = [None] * NHL

                def pv_h01(kt, ex):
                    off = koff(J, kt)
                    nc.tensor.matmul(
                        yT0[:, off:512], vaug[:, kt, 0:65], ex[:, off:512],
                        start=(kt == 0), stop=(kt == nkt - 1),
                    )
                    nc.tensor.matmul(
                        yT1[:, off:512], vaug[:, kt, 65:130],
                        ex[:, 512 + off : 1024],
                        start=(kt == 0), stop=(kt == nkt - 1),
                    )

                yT0 = psY.tile([65, 512], f32, tag="yt")
                yT1 = psY.tile([65, 512], f32, tag="yt")
                pp = pending_proj
                pending_proj = []
                ppi = 0
                ex_p = scores_exp_h01(J, 0)
                for kt in range(1, nkt):
                    ex = scores_exp_h01(J, kt)
                    pv_h01(kt - 1, ex_p)
                    ex_p = ex
                    want = kt * len(pp) // max(nkt - 1, 1)
                    while ppi < want:
                        pp[ppi]()
                        ppi += 1
                pv_h01(nkt - 1, ex_p)
                while ppi < len(pp):
                    pp[ppi]()
                    ppi += 1
                finalize(yT0, yts_l, 0)
                finalize(yT1, yts_l, 1)
                qparts = qkv_parts(J + 1) if J + 1 < QB else []

                # head 2: ktile pairs [kt_a | kt_b], same pipelining
                def pv_h2(g, ex):
                    offa, offb = koff(J, 2 * g), koff(J, 2 * g + 1)
                    nc.tensor.matmul(
                        yT2[:, offa:512], vaug[:, 2 * g, ds(2 * 65, 65)],
                        ex[:, offa:512],
                        start=(g == 0), stop=False,
                    )
                    nc.tensor.matmul(
                        yT2[:, offb:512], vaug[:, 2 * g + 1, ds(2 * 65, 65)],
                        ex[:, 512 + offb : 1024],
                        start=False, stop=(g == nkt // 2 - 1),
                    )

                yT2 = psY.tile([65, 512], f32, tag="yt")
                qpi = 0
                npair = nkt // 2
                ex_p = scores_exp_h2(J, 0)
                for g in range(1, npair):
                    ex = scores_exp_h2(J, g)
                    pv_h2(g - 1, ex_p)
                    ex_p = ex
                    want = g * len(qparts) // max(npair - 1, 1)
                    while qpi < want:
                        qparts[qpi]()
                        qpi += 1
                pv_h2(npair - 1, ex_p)
                while qpi < len(qparts):
                    qparts[qpi]()
                    qpi += 1
                finalize(yT2, yts_l, 2)
                pending_proj = proj_parts(J, yts_l)
            for part in pending_proj:
                part()
    nc.compile()
    return nc


def _host_masks():
    import ml_dtypes

    p = np.arange(128)[:, None]
    f = np.arange(512)[None, :]
    m = np.zeros((128, 4, 512), np.float32)
    for j in range(4):
        m[:, j, :] = (f >= p + 128 * j).astype(np.float32)
    pairs = m.reshape(128, 2048).astype(ml_dtypes.bfloat16)
    dup = np.concatenate([m, m], axis=2).reshape(128, 4096)  # [m_j | m_j] per j
    return pairs, dup.astype(ml_dtypes.bfloat16)


def _core_inputs(x, w_attn, b_attn, hg, t_len, has_ba):
    # column ranges of this head-group inside w_attn: q | k | v blocks of C each
    q0 = 192 * hg
    wqk = np.zeros((C, 3, 128), np.float32)
    wqk[:, 0, 0:64] = w_attn[:, q0 : q0 + 64]                 # q0
    wqk[:, 0, 64:128] = w_attn[:, q0 + 64 : q0 + 128]         # q1
    wqk[:, 1, 0:64] = w_attn[:, C + q0 : C + q0 + 64]         # k0
    wqk[:, 1, 64:128] = w_attn[:, C + q0 + 64 : C + q0 + 128] # k1
    wqk[:, 2, 0:64] = w_attn[:, q0 + 128 : q0 + 192]          # q2
    wqk[:, 2, 64:128] = w_attn[:, C + q0 + 128 : C + q0 + 192]  # k2
    mp, mh = _host_masks()
    ins = {
        "x": np.ascontiguousarray(x),
        "wqk": np.ascontiguousarray(wqk.reshape(C, 384)),
        "wv": np.ascontiguousarray(w_attn[:, 2 * C + q0 : 2 * C + q0 + 192]),
        "masks": mp,
        "maskshh": mh,
    }
    if has_ba:
        baqk = np.zeros((1, 384), np.float32)
        baqk[0, 0:64] = b_attn[q0 : q0 + 64]
        baqk[0, 64:128] = b_attn[q0 + 64 : q0 + 128]
        baqk[0, 128:192] = b_attn[C + q0 : C + q0 + 64]
        baqk[0, 192:256] = b_attn[C + q0 + 64 : C + q0 + 128]
        baqk[0, 256:320] = b_attn[q0 + 128 : q0 + 192]
        baqk[0, 320:384] = b_attn[C + q0 + 128 : C + q0 + 192]
        ins["baqk"] = baqk
        ins["bav"] = np.ascontiguousarray(
            b_attn[2 * C + q0 : 2 * C + q0 + 192].reshape(1, 192)
        )
    return ins


def kernel(**inputs):
    from concourse.bass_utils import run_bass_kernel_spmd

    x = np.asarray(inputs["x"], dtype=np.float32)
    w_attn = np.asarray(inputs["w_attn"], dtype=np.float32)
    b_attn = np.asarray(inputs["b_attn"], dtype=np.float32)
    w_proj = np.asarray(inputs["w_proj"], dtype=np.float32)
    b_proj = np.asarray(inputs["b_proj"], dtype=np.float32)

    has_ba = bool(np.any(b_attn))
    key = (T, has_ba)
    if key not in _PROG_CACHE:
        _PROG_CACHE[key] = _build(T, has_ba)
    nc = _PROG_CACHE[key]

    in_maps = []
    for c in range(8):
        b, hg = c // 4, c % 4
        ins = _core_inputs(x[b], w_attn, b_attn, hg, T, has_ba)
        ins["wp"] = np.ascontiguousarray(w_proj[192 * hg : 192 * hg + 192, :])
        in_maps.append(ins)

    res = run_bass_kernel_spmd(nc, in_maps, core_ids=list(range(8)), trace=TRACE)
    global LAST
    LAST = res
    out = np.zeros((B, T, C), np.float32)
    for c in range(8):
        out[c // 4] += res.results[c]["out"]
    out += b_proj
    return out





# revision 11
# speedup vs baseline: 1.0660x; 1.0660x over previous
# Causal self-attention (B=2, T=4096, C=768, H=12, D=64) on 8 trn2 cores.
#
# Sharding: core c = 4*b + hg handles batch b and head-group hg (3 heads).
# Per core:
#   x arrives host-transposed/bf16 (xT [C,T]); QKV matmuls read it directly
#   (no on-device transposes). qT,kT (d-major) and v (t-major) in SBUF.
#   flash-style causal attention: scoresT (tk partitions, tq free) -> exp on
#   ACT -> PV in [q, d] orientation (65-wide streams incl. a ones column for
#   the softmax sums). Normalize = per-partition reciprocal * evac, then DMA
#   transpose to d-major and a K=128 head-stacked projection. Host sums the
#   4 head-group partials per batch.
import numpy as np

B, T, C = 2, 4096, 768
H, D = 12, 64
NHL = 3          # heads per core
CT = C // 128    # 6 contraction tiles

_PROG_CACHE = {}

# test-harness hooks (harmless when unused): set TRACE=True before calling
# kernel() to capture an NTFF profile; the BassKernelResults lands in LAST.
TRACE = False
LAST = None


def _build(t_len, has_ba):
    import concourse.bass as bass
    import concourse.bacc as bacc
    import concourse.mybir as mybir
    import concourse.tile as tile
    from concourse.bass import ts, ds

    f32 = mybir.dt.float32
    bf16 = mybir.dt.bfloat16
    AF = mybir.ActivationFunctionType
    ALU = mybir.AluOpType

    TB = t_len // 128   # 128-row t tiles
    QB = t_len // 512   # 512-row q blocks

    nc = bacc.Bacc("TRN2", target_bir_lowering=False, debug=False)

    xT_d = nc.dram_tensor("xT", [C, t_len], bf16, kind="ExternalInput").ap()
    wqk_d = nc.dram_tensor("wqk", [C, 384], f32, kind="ExternalInput").ap()
    wv_d = nc.dram_tensor("wv", [C, NHL * D], f32, kind="ExternalInput").ap()
    wp01_d = nc.dram_tensor("wp01", [128, C], f32, kind="ExternalInput").ap()
    wp2_d = nc.dram_tensor("wp2", [128, C], f32, kind="ExternalInput").ap()
    masks_d = nc.dram_tensor("masks", [128, 2048], bf16, kind="ExternalInput").ap()
    maskshh_d = nc.dram_tensor(
        "maskshh", [128, 4096], bf16, kind="ExternalInput"
    ).ap()
    if has_ba:
        baqk_d = nc.dram_tensor("baqk", [1, 384], f32, kind="ExternalInput").ap()
        bav_d = nc.dram_tensor("bav", [1, NHL * D], f32, kind="ExternalInput").ap()
    out_d = nc.dram_tensor("out", [t_len, C], f32, kind="ExternalOutput").ap()

    # psV slot map (uniform 6+6, 65-wide slots, sums ride at col +64):
    # bank0 slots k=0..5 at col 65k: h01 accums a01=2s+h for a01<6;
    # bank1 slots at col 512+65k: k=0,1 -> a01 6,7 (s=3), k=2+s -> h2 sub s.
    def pvcol(s, h):
        if h < 2:
            a = 2 * s + h
            return 65 * a if a < 6 else 512 + 65 * (a - 6)
        return 512 + 65 * (2 + s)

    with tile.TileContext(nc) as tc:
        with (
            tc.tile_pool(name="const", bufs=1) as constp,
            tc.tile_pool(name="big", bufs=1) as bigp,
            tc.tile_pool(name="xTp", bufs=2) as xTp,
            tc.tile_pool(name="expp", bufs=8) as expp,
            tc.tile_pool(name="yts01p", bufs=2) as yts01p,
            tc.tile_pool(name="yts2p", bufs=2) as yts2p,
            tc.tile_pool(name="yT01p", bufs=2) as yT01p,
            tc.tile_pool(name="yT2p", bufs=2) as yT2p,
            tc.tile_pool(name="sumsp", bufs=2) as sumsp,
            tc.tile_pool(name="recp", bufs=2) as recp,
            tc.tile_pool(name="accp", bufs=3) as accp,
            tc.tile_pool(name="psA", bufs=2, space="PSUM") as psA,
            tc.tile_pool(name="psV", bufs=1, space="PSUM") as psVp,
            tc.tile_pool(name="psQ", bufs=2, space="PSUM") as psQ,
        ):
            # ---- persistent SBUF tensors ----
            qkT = bigp.tile([128, 4, t_len], bf16)        # [q0|q1],[k0|k1],[q2|k2],[k2|-]
            vaug = bigp.tile([128, TB, NHL * 65], bf16)   # v + ones col per head
            wqk_sb = bigp.tile([128, CT, 384], bf16)
            wv_sb = bigp.tile([128, CT, NHL * D], bf16)
            wp01_sb = bigp.tile([128, C], bf16)
            wp2_sb = bigp.tile([128, C], bf16)
            masks_sb = bigp.tile([128, 2048], bf16)
            maskshh_sb = bigp.tile([128, 4096], bf16)
            if has_ba:
                baqk_sb = constp.tile([1, 384], bf16)
                bav_sb = constp.tile([1, NHL * D], bf16)
                ones128 = constp.tile([1, 128], bf16)
                ones512 = constp.tile([1, 512], bf16)
                nc.vector.memset(ones128, 1.0)
                nc.vector.memset(ones512, 1.0)
                nc.gpsimd.dma_start(out=baqk_sb, in_=baqk_d)
                nc.gpsimd.dma_start(out=bav_sb, in_=bav_d)

            nc.gpsimd.dma_start(
                out=wqk_sb, in_=wqk_d.rearrange("(ct p) n -> p ct n", p=128)
            )
            nc.gpsimd.dma_start(
                out=wv_sb, in_=wv_d.rearrange("(ct p) n -> p ct n", p=128)
            )
            nc.gpsimd.dma_start(out=wp01_sb, in_=wp01_d)
            nc.gpsimd.dma_start(out=wp2_sb, in_=wp2_d)

            # ones columns of vaug (col 64 of each head's 65-wide chunk)
            vaug4 = vaug.rearrange("p t (h e) -> p t h e", e=65)
            for h in range(NHL):
                nc.vector.memset(vaug4[:, :, h, 64:65], 1.0)

            # scores operand slices: head -> (partition offset, qk tile indices)
            def q_ap(h, J):
                p0, nt = [(0, 0), (64, 0), (0, 2)][h]
                return qkT[p0 : p0 + 64, nt, ts(J, 512)]

            def k_ap(h, kt):
                p0, nt = [(0, 1), (64, 1), (0, 3)][h]
                return qkT[p0 : p0 + 64, nt, ts(kt, 128)]

            xT_r = xT_d.rearrange("(ct p) t -> p ct t", p=128)

            def qkv_parts(J):
                """Emit block J's xT load now; return 8 closures (4 qk + 4 v
                matmul groups) to be interleaved into other loops."""
                xTb = xTp.tile([128, CT, 512], bf16, tag="xT")
                nc.gpsimd.dma_start(out=xTb, in_=xT_r[:, :, ts(J, 512)])

                def qk_group(nt):
                    qk_ps = psQ.tile([128, 512], f32, tag="pq")
                    for ct in range(CT):
                        nc.tensor.matmul(
                            qk_ps,
                            wqk_sb[:, ct, ts(nt, 128)],
                            xTb[:, ct, :],
                            start=(ct == 0),
                            stop=(ct == CT - 1 and not has_ba),
                        )
                    if has_ba:
                        nc.tensor.matmul(
                            qk_ps, baqk_sb[:, ts(nt, 128)], ones512,
                            start=False, stop=True,
                        )
                    nc.vector.tensor_copy(
                        out=qkT[:, nt, ts(J, 512)], in_=qk_ps
                    )

                def v_group(sub):
                    tb = J * 4 + sub
                    v_ps = psQ.tile([128, 512], f32, tag="pq")
                    for ct in range(CT):
                        nc.tensor.matmul(
                            v_ps[:, 0 : NHL * D],
                            xTb[:, ct, ts(sub, 128)],
                            wv_sb[:, ct, :],
                            start=(ct == 0),
                            stop=(ct == CT - 1 and not has_ba),
                        )
                    if has_ba:
                        nc.tensor.matmul(
                            v_ps[:, 0 : NHL * D], ones128, bav_sb,
                            start=False, stop=True,
                        )
                    nc.vector.tensor_copy(
                        out=vaug4[:, tb, :, 0:64],
                        in_=v_ps[:, 0 : NHL * D].rearrange(
                            "p (h e) -> p h e", e=64
                        ),
                    )

                def k2_shift():
                    # nt3[0:64] = k2 (from nt2's upper half) so h2's scores
                    # get base-aligned operands without a 4th matmul group
                    nc.gpsimd.dma_start(
                        out=qkT[0:64, 3, ts(J, 512)],
                        in_=qkT[64:128, 2, ts(J, 512)],
                    )

                return (
                    [lambda nt=nt: qk_group(nt) for nt in (0, 1)]
                    + [lambda sub=sub: v_group(sub) for sub in range(4)]
                    + [lambda: qk_group(2), k2_shift]
                )

            def qkv_block(J):
                for part in qkv_parts(J):
                    part()

            def koff(Jq, kt):
                # first valid q column for (possibly diagonal) ktile kt
                return 128 * (kt - 4 * Jq) if kt >= 4 * Jq else 0

            def scores_exp_h01(Jq, kt):
                J = Jq
                off = koff(Jq, kt)
                sc = psA.tile([128, 1024], f32, tag="psa")
                nc.tensor.matmul(
                    sc[:, off:512], k_ap(0, kt), q_ap(0, J)[:, off:512],
                    start=True, stop=True,
                )
                nc.tensor.matmul(
                    sc[:, 512 + off : 1024], k_ap(1, kt),
                    q_ap(1, J)[:, off:512],
                    start=True, stop=True,
                )
                ex = expp.tile([128, 1024], bf16, tag="ex")
                sc2 = sc.rearrange("p (two n) -> p two n", two=2)
                ex2 = ex.rearrange("p (two n) -> p two n", two=2)
                nc.scalar.activation(
                    ex2[:, :, off:512], sc2[:, :, off:512], AF.Exp, scale=0.125
                )
                if kt >= 4 * J:
                    m4 = maskshh_sb.rearrange(
                        "p (j two n) -> p j two n", j=4, two=2
                    )
                    nc.vector.tensor_mul(
                        ex2[:, :, off:512],
                        ex2[:, :, off:512],
                        m4[:, kt - 4 * J, :, off:512],
                    )
                return ex

            def scores_exp_h2(Jq, g):
                J = Jq
                offa, offb = koff(Jq, 2 * g), koff(Jq, 2 * g + 1)
                sc = psA.tile([128, 1024], f32, tag="psa")
                nc.tensor.matmul(
                    sc[:, offa:512], k_ap(2, 2 * g), q_ap(2, J)[:, offa:512],
                    start=True, stop=True,
                )
                nc.tensor.matmul(
                    sc[:, 512 + offb : 1024], k_ap(2, 2 * g + 1),
                    q_ap(2, J)[:, offb:512],
                    start=True, stop=True,
                )
                ex = expp.tile([128, 1024], bf16, tag="ex")
                if offa == offb:
                    sc2 = sc.rearrange("p (two n) -> p two n", two=2)
                    ex2 = ex.rearrange("p (two n) -> p two n", two=2)
                    nc.scalar.activation(
                        ex2[:, :, offa:512], sc2[:, :, offa:512],
                        AF.Exp, scale=0.125,
                    )
                else:
                    nc.scalar.activation(
                        ex[:, offa:512], sc[:, offa:512], AF.Exp, scale=0.125
                    )
                    nc.scalar.activation(
                        ex[:, 512 + offb : 1024], sc[:, 512 + offb : 1024],
                        AF.Exp, scale=0.125,
                    )
                if g >= 2 * Jq:
                    ja, jb = 2 * g - 4 * Jq, 2 * g + 1 - 4 * Jq
                    m4 = masks_sb.rearrange("p (j n) -> p j n", j=4)
                    nc.vector.tensor_mul(
                        ex[:, offa:512],
                        ex[:, offa:512],
                        m4[:, ja, offa:512],
                    )
                    nc.vector.tensor_mul(
                        ex[:, 512 + offb : 1024],
                        ex[:, 512 + offb : 1024],
                        m4[:, jb, offb:512],
                    )
                return ex

            # ---- pipelined main loop ----
            qkv_block(0)
            # mask loads emitted after block-0 QKV so the serial DMA engines
            # prioritize x/wqk (masks are first read ~20us in, by J=0's exps)
            nc.sync.dma_start(out=masks_sb, in_=masks_d)
            nc.sync.dma_start(out=maskshh_sb, in_=maskshh_d)
            pending_proj = []
            for J in range(QB):
                nkt = 4 * (J + 1)
                psV = psVp.tile([128, 1024], f32, tag="psv")

                def pv_h01(kt, ex):
                    s0 = max(0, kt - 4 * J)
                    for s in range(s0, 4):
                        for h in (0, 1):
                            c = pvcol(s, h)
                            # bank0 group opens at (0,0,0), closes at
                            # (nkt-2,2,1); bank1 opens at (0,3,0), closes in h2
                            st = kt == 0 and ((s == 0 and h == 0) or (s == 3 and h == 0))
                            sp = kt == nkt - 2 and s == 2 and h == 1
                            nc.tensor.matmul(
                                psV[:, c : c + 65],
                                ex[:, 512 * h + 128 * s : 512 * h + 128 * (s + 1)],
                                vaug4[:, kt, h, :],
                                start=st, stop=sp,
                            )

                def pv_h2(g, ex):
                    for kt, base in ((2 * g, 0), (2 * g + 1, 512)):
                        s0 = max(0, kt - 4 * J)
                        for s in range(s0, 4):
                            c = pvcol(s, 2)
                            sp = kt == nkt - 1 and s == 3
                            nc.tensor.matmul(
                                psV[:, c : c + 65],
                                ex[:, base + 128 * s : base + 128 * (s + 1)],
                                vaug4[:, kt, 2, :],
                                start=False, stop=sp,
                            )

                sums = sumsp.tile([128, 16], f32, tag="sums")
                rec = recp.tile([128, 16], f32, tag="rec")
                yts01 = yts01p.tile([128, 4, 128], bf16, tag="yts01")
                yts2 = yts2p.tile([128, 2, 128], bf16, tag="yts2")
                yT01 = yT01p.tile([128, 4, 128], bf16, tag="yT01")
                yT2 = yT2p.tile([128, 2, 128], bf16, tag="yT2")

                def finalize_h01():
                    # gather the 8 h01 sums (6 in bank0, 2 in bank1), one
                    # reciprocal, then two broadcast-normalize evacs
                    psV0 = psV[:, 0:390].rearrange("p (k e) -> p k e", e=65)
                    psV1 = psV[:, 512:642].rearrange("p (k e) -> p k e", e=65)
                    nc.vector.tensor_copy(
                        out=sums[:, 0:6],
                        in_=psV0[:, :, 64:65].rearrange("p k e -> p (k e)"),
                    )
                    nc.vector.tensor_copy(
                        out=sums[:, 6:8],
                        in_=psV1[:, :, 64:65].rearrange("p k e -> p (k e)"),
                    )
                    nc.vector.reciprocal_approx_fast(rec[:, 0:8], sums[:, 0:8])
                    yts01f = yts01.rearrange("p s x -> p (s x)")
                    nc.vector.tensor_tensor(
                        out=yts01f[:, 0:384].rearrange("p (k e) -> p k e", e=64),
                        in0=psV0[:, :, 0:64],
                        in1=rec[:, 0:6].unsqueeze(-1).broadcast_to([128, 6, 64]),
                        op=ALU.mult,
                    )
                    nc.vector.tensor_tensor(
                        out=yts01f[:, 384:512].rearrange(
                            "p (k e) -> p k e", e=64
                        ),
                        in0=psV1[:, :, 0:64],
                        in1=rec[:, 6:8].unsqueeze(-1).broadcast_to([128, 2, 64]),
                        op=ALU.mult,
                    )
                    for s in range(4):
                        nc.sync.dma_start(
                            out=yT01[:, s, :], in_=yts01[:, s, :],
                            transpose=True,
                        )

                def finalize_h2():
                    psV2 = psV[:, 642:902].rearrange("p (k e) -> p k e", e=65)
                    nc.vector.tensor_copy(
                        out=sums[:, 8:12],
                        in_=psV2[:, :, 64:65].rearrange("p k e -> p (k e)"),
                    )
                    nc.vector.reciprocal_approx_fast(rec[:, 8:12], sums[:, 8:12])
                    nc.vector.tensor_tensor(
                        out=yts2.rearrange("p s x -> p (s x)").rearrange(
                            "p (k e) -> p k e", e=64
                        ),
                        in0=psV2[:, :, 0:64],
                        in1=rec[:, 8:12].unsqueeze(-1).broadcast_to([128, 4, 64]),
                        op=ALU.mult,
                    )
                    for p in range(2):
                        nc.sync.dma_start(
                            out=yT2[:, p, :], in_=yts2[:, p, :], transpose=True
                        )

                # --- h01 phase, pipelined: scores(kt+1) ahead of PV(kt),
                # proj(J-1) parts drained proportionally ---
                pp = pending_proj
                pending_proj = []
                ppi = 0
                ex_p = scores_exp_h01(J, 0)
                for kt in range(1, nkt):
                    ex = scores_exp_h01(J, kt)
                    pv_h01(kt - 1, ex_p)
                    ex_p = ex
                    want = kt * len(pp) // max(nkt - 1, 1)
                    while ppi < want:
                        pp[ppi]()
                        ppi += 1
                pv_h01(nkt - 1, ex_p)
                finalize_h01()
                while ppi < len(pp):
                    pp[ppi]()
                    ppi += 1
                qparts = qkv_parts(J + 1) if J + 1 < QB else []

                # --- h2 phase: ktile pairs, qkv(J+1) parts drained ---
                qpi = 0
                npair = nkt // 2
                ex_p = scores_exp_h2(J, 0)
                for g in range(1, npair):
                    ex = scores_exp_h2(J, g)
                    pv_h2(g - 1, ex_p)
                    ex_p = ex
                    want = g * len(qparts) // max(npair - 1, 1)
                    while qpi < want:
                        qparts[qpi]()
                        qpi += 1
                pv_h2(npair - 1, ex_p)
                finalize_h2()
                while qpi < len(qparts):
                    qparts[qpi]()
                    qpi += 1

                # --- projection closures for this J (run during J+1) ---
                def proj_parts(J, yT01, yT2):
                    def jj_part(jj):
                        p, half = jj // 2, jj % 2
                        y2 = yT2[64 * half : 64 * half + 64, p, :]
                        w2 = wp2_sb[64 * half : 64 * half + 64, :]
                        pj = psQ.tile([128, 512], f32, tag="pq")
                        nc.tensor.matmul(
                            pj, yT01[:, jj, :], wp01_sb[:, 0:512],
                            start=True, stop=False,
                        )
                        nc.tensor.matmul(
                            pj, y2, w2[:, 0:512], start=False, stop=True
                        )
                        acc = accp.tile([128, C], f32, tag="acc")
                        nc.vector.tensor_copy(out=acc[:, 0:512], in_=pj)
                        pj2 = psQ.tile([128, 512], f32, tag="pq")
                        nc.tensor.matmul(
                            pj2[:, 0:256], yT01[:, jj, :], wp01_sb[:, 512:768],
                            start=True, stop=False,
                        )
                        nc.tensor.matmul(
                            pj2[:, 0:256], y2, w2[:, 512:768],
                            start=False, stop=True,
                        )
                        nc.vector.tensor_copy(
                            out=acc[:, 512:768], in_=pj2[:, 0:256]
                        )
                        nc.sync.dma_start(
                            out=out_d[ds(J * 512 + jj * 128, 128), :], in_=acc
                        )

                    return [lambda jj=jj: jj_part(jj) for jj in range(4)]

                pending_proj = proj_parts(J, yT01, yT2)
            for part in pending_proj:
                part()
    nc.compile()
    return nc


def _host_masks():
    import ml_dtypes

    p = np.arange(128)[:, None]
    f = np.arange(512)[None, :]
    m = np.zeros((128, 4, 512), np.float32)
    for j in range(4):
        m[:, j, :] = (f >= p + 128 * j).astype(np.float32)
    pairs = m.reshape(128, 2048).astype(ml_dtypes.bfloat16)
    dup = np.concatenate([m, m], axis=2).reshape(128, 4096)  # [m_j | m_j] per j
    return pairs, dup.astype(ml_dtypes.bfloat16)


def _core_inputs(xT, w_attn, b_attn, hg, t_len, has_ba):
    # column ranges of this head-group inside w_attn: q | k | v blocks of C each
    q0 = 192 * hg
    wqk = np.zeros((C, 3, 128), np.float32)
    wqk[:, 0, 0:64] = w_attn[:, q0 : q0 + 64]                 # q0
    wqk[:, 0, 64:128] = w_attn[:, q0 + 64 : q0 + 128]         # q1
    wqk[:, 1, 0:64] = w_attn[:, C + q0 : C + q0 + 64]         # k0
    wqk[:, 1, 64:128] = w_attn[:, C + q0 + 64 : C + q0 + 128] # k1
    wqk[:, 2, 0:64] = w_attn[:, q0 + 128 : q0 + 192]          # q2
    wqk[:, 2, 64:128] = w_attn[:, C + q0 + 128 : C + q0 + 192]  # k2
    mp, mh = _host_masks()
    ins = {
        "xT": xT,
        "wqk": np.ascontiguousarray(wqk.reshape(C, 384)),
        "wv": np.ascontiguousarray(w_attn[:, 2 * C + q0 : 2 * C + q0 + 192]),
        "masks": mp,
        "maskshh": mh,
    }
    if has_ba:
        baqk = np.zeros((1, 384), np.float32)
        baqk[0, 0:64] = b_attn[q0 : q0 + 64]
        baqk[0, 64:128] = b_attn[q0 + 64 : q0 + 128]
        baqk[0, 128:192] = b_attn[C + q0 : C + q0 + 64]
        baqk[0, 192:256] = b_attn[C + q0 + 64 : C + q0 + 128]
        baqk[0, 256:320] = b_attn[q0 + 128 : q0 + 192]
        baqk[0, 320:384] = b_attn[C + q0 + 128 : C + q0 + 192]
        ins["baqk"] = baqk
        ins["bav"] = np.ascontiguousarray(
            b_attn[2 * C + q0 : 2 * C + q0 + 192].reshape(1, 192)
        )
    return ins


def kernel(**inputs):
    import ml_dtypes
    from concourse.bass_utils import run_bass_kernel_spmd

    x = np.asarray(inputs["x"], dtype=np.float32)
    w_attn = np.asarray(inputs["w_attn"], dtype=np.float32)
    b_attn = np.asarray(inputs["b_attn"], dtype=np.float32)
    w_proj = np.asarray(inputs["w_proj"], dtype=np.float32)
    b_proj = np.asarray(inputs["b_proj"], dtype=np.float32)

    has_ba = bool(np.any(b_attn))
    key = (T, has_ba)
    if key not in _PROG_CACHE:
        _PROG_CACHE[key] = _build(T, has_ba)
    nc = _PROG_CACHE[key]

    xTs = [
        np.ascontiguousarray(x[b].T).astype(ml_dtypes.bfloat16)
        for b in range(B)
    ]
    in_maps = []
    for c in range(8):
        b, hg = c // 4, c % 4
        ins = _core_inputs(xTs[b], w_attn, b_attn, hg, T, has_ba)
        wp = w_proj[192 * hg : 192 * hg + 192, :]
        ins["wp01"] = np.ascontiguousarray(wp[0:128, :])
        ins["wp2"] = np.ascontiguousarray(
            np.concatenate([wp[128:192, :], wp[128:192, :]], axis=0)
        )
        in_maps.append(ins)

    res = run_bass_kernel_spmd(nc, in_maps, core_ids=list(range(8)), trace=TRACE)
    global LAST
    LAST = res
    out = np.zeros((B, T, C), np.float32)
    for c in range(8):
        out[c // 4] += res.results[c]["out"]
    out += b_proj
    return out


# revision 29
# speedup vs baseline: 1.1184x; 1.0492x over previous
# Causal self-attention (B=2, T=4096, C=768, H=12, D=64) on 8 trn2 cores.
#
# Sharding: core c = 4*b + hg handles batch b and head-group hg (3 heads).
# Per core:
#   x arrives host-transposed/bf16 (xT [C,T]); QKV matmuls read it directly
#   (no on-device transposes). qT,kT (d-major) and v (t-major) in SBUF.
#   flash-style causal attention: scoresT (tk partitions, tq free) -> exp on
#   ACT -> PV in [q, d] orientation (65-wide streams incl. a ones column for
#   the softmax sums). Normalize = per-partition reciprocal * evac, then DMA
#   transpose to d-major and a K=128 head-stacked projection. Host sums the
#   4 head-group partials per batch.
import numpy as np

B, T, C = 2, 4096, 768
H, D = 12, 64
NHL = 3          # heads per core
CT = C // 128    # 6 contraction tiles

_PROG_CACHE = {}
SCHRAUD = True

# test-harness hooks (harmless when unused): set TRACE=True before calling
# kernel() to capture an NTFF profile; the BassKernelResults lands in LAST.
TRACE = False
LAST = None


def _build(t_len, has_ba):
    import concourse.bass as bass
    import concourse.bacc as bacc
    import concourse.mybir as mybir
    import concourse.tile as tile
    from concourse.bass import ts, ds

    f32 = mybir.dt.float32
    bf16 = mybir.dt.bfloat16
    i16 = mybir.dt.int16
    AF = mybir.ActivationFunctionType
    ALU = mybir.AluOpType

    # Schraudolph exp in bf16-bit space on DVE: bits(exp(s*0.125)) ~=
    # round(A*s + BC) as int16, then bitcast to bf16. A = 2^7*log2(e)/8;
    # BC = 127*2^7 - 5.6 (mean-error-centering offset).
    SCH_A = 128.0 * 1.4426950408889634 * 0.125
    SCH_B = 16256.0 - 5.6

    def dve_exp(ex_bf, sc_f32):
        nc.vector.tensor_scalar(
            out=ex_bf.bitcast(i16),
            in0=sc_f32,
            scalar1=SCH_A,
            scalar2=SCH_B,
            op0=ALU.mult,
            op1=ALU.add,
        )

    TB = t_len // 128   # 128-row t tiles
    QB = t_len // 512   # 512-row q blocks

    nc = bacc.Bacc("TRN2", target_bir_lowering=False, debug=False)

    xT_d = nc.dram_tensor("xT", [C, t_len], bf16, kind="ExternalInput").ap()
    wqk_d = nc.dram_tensor("wqk", [C, 384], bf16, kind="ExternalInput").ap()
    wv_d = nc.dram_tensor("wv", [C, NHL * D], bf16, kind="ExternalInput").ap()
    wp01_d = nc.dram_tensor("wp01", [128, C], bf16, kind="ExternalInput").ap()
    wp2_d = nc.dram_tensor("wp2", [128, C], bf16, kind="ExternalInput").ap()
    masks_d = nc.dram_tensor("masks", [128, 2048], bf16, kind="ExternalInput").ap()
    ident_d = nc.dram_tensor("ident", [128, 128], bf16, kind="ExternalInput").ap()
    maskshh_d = nc.dram_tensor(
        "maskshh", [128, 4096], bf16, kind="ExternalInput"
    ).ap()
    if has_ba:
        baqk_d = nc.dram_tensor("baqk", [1, 384], f32, kind="ExternalInput").ap()
        bav_d = nc.dram_tensor("bav", [1, NHL * D], f32, kind="ExternalInput").ap()
    out_d = nc.dram_tensor("out", [t_len, C], bf16, kind="ExternalOutput").ap()

    # psV slot map (uniform 6+6, 65-wide slots, sums ride at col +64):
    # bank0 slots k=0..5 at col 65k: h01 accums a01=2s+h for a01<6;
    # bank1 slots at col 512+65k: k=0,1 -> a01 6,7 (s=3), k=2+s -> h2 sub s.
    def pvcol(s, h):
        if h < 2:
            a = 2 * s + h
            return 65 * a if a < 6 else 512 + 65 * (a - 6)
        return 512 + 65 * (2 + s)

    with tile.TileContext(nc) as tc:
        with (
            tc.tile_pool(name="const", bufs=1) as constp,
            tc.tile_pool(name="big", bufs=1) as bigp,
            tc.tile_pool(name="xTp", bufs=3) as xTp,
            tc.tile_pool(name="expp", bufs=8) as expp,
            tc.tile_pool(name="yts01p", bufs=2) as yts01p,
            tc.tile_pool(name="yts2p", bufs=2) as yts2p,
            tc.tile_pool(name="yT01p", bufs=2) as yT01p,
            tc.tile_pool(name="yT2p", bufs=2) as yT2p,
            tc.tile_pool(name="sumsp", bufs=2) as sumsp,
            tc.tile_pool(name="recp", bufs=2) as recp,
            tc.tile_pool(name="accp", bufs=3) as accp,
            tc.tile_pool(name="psA", bufs=3, space="PSUM") as psA,
            tc.tile_pool(name="psV", bufs=1, space="PSUM") as psVp,
        ):
            # ---- persistent SBUF tensors ----
            qkT = bigp.tile([128, 4, t_len], bf16)        # [q0|q1],[k0|k1],[q2|k2],[k2|-]
            vaug = bigp.tile([128, TB, NHL * 65], bf16)   # v + ones col per head
            wqk_sb = bigp.tile([128, CT, 384], bf16)
            wv_sb = bigp.tile([128, CT, NHL * D], bf16)
            wp01_sb = bigp.tile([128, C], bf16)
            wp2_sb = bigp.tile([128, C], bf16)
            masks_sb = bigp.tile([128, 2048], bf16)
            ident_sb = bigp.tile([128, 128], bf16)
            maskshh_sb = bigp.tile([128, 4096], bf16)
            if has_ba:
                baqk_sb = constp.tile([1, 384], bf16)
                bav_sb = constp.tile([1, NHL * D], bf16)
                ones128 = constp.tile([1, 128], bf16)
                ones512 = constp.tile([1, 512], bf16)
                nc.vector.memset(ones128, 1.0)
                nc.vector.memset(ones512, 1.0)
                nc.gpsimd.dma_start(out=baqk_sb, in_=baqk_d)
                nc.gpsimd.dma_start(out=bav_sb, in_=bav_d)

            # wqk + x(0) gate the first matmuls: issue them on the two
            # fast HWDGE queues; everything else trails on gpsimd/swdge
            nc.sync.dma_start(
                out=wqk_sb, in_=wqk_d.rearrange("(ct p) n -> p ct n", p=128)
            )

            # ones columns of vaug (col 64 of each head's 65-wide chunk)
            vaug4 = vaug.rearrange("p t (h e) -> p t h e", e=65)
            for h in range(NHL):
                nc.vector.memset(vaug4[:, :, h, 64:65], 1.0)

            # scores operand slices: head -> (partition offset, qk tile indices)
            def q_ap(h, J):
                p0, nt = [(0, 0), (64, 0), (0, 2)][h]
                return qkT[p0 : p0 + 64, nt, ts(J, 512)]

            def k_ap(h, kt):
                p0, nt = [(0, 1), (64, 1), (0, 3)][h]
                return qkT[p0 : p0 + 64, nt, ts(kt, 128)]

            xT_r = xT_d.rearrange("(ct p) t -> p ct t", p=128)
            xT_tiles = {}

            def prefetch_x(J):
                if J < QB:
                    xTb = xTp.tile([128, CT, 512], bf16, tag="xT")
                    nc.gpsimd.dma_start(out=xTb, in_=xT_r[:, :, ts(J, 512)])
                    xT_tiles[J] = xTb

            def qkv_parts(J):
                """Return 8 closures (4 qk + 4 v matmul groups) over the
                prefetched xT block, to be interleaved into other loops."""
                xTb = xT_tiles.pop(J)

                def qk_group(nt):
                    qk_pt = psA.tile([128, 1024], f32, tag="psa")
                    qk_ps = qk_pt[:, 0:512]
                    for ct in range(CT):
                        nc.tensor.matmul(
                            qk_ps,
                            wqk_sb[:, ct, ts(nt, 128)],
                            xTb[:, ct, :],
                            start=(ct == 0),
                            stop=(ct == CT - 1 and not has_ba),
                        )
                    if has_ba:
                        nc.tensor.matmul(
                            qk_ps, baqk_sb[:, ts(nt, 128)], ones512,
                            start=False, stop=True,
                        )
                    nc.vector.tensor_copy(
                        out=qkT[:, nt, ts(J, 512)], in_=qk_ps
                    )

                def v_group(sub):
                    tb = J * 4 + sub
                    v_pt = psA.tile([128, 1024], f32, tag="psa")
                    v_ps = v_pt[:, 0:512]
                    for ct in range(CT):
                        nc.tensor.matmul(
                            v_ps[:, 0 : NHL * D],
                            xTb[:, ct, ts(sub, 128)],
                            wv_sb[:, ct, :],
                            start=(ct == 0),
                            stop=(ct == CT - 1 and not has_ba),
                        )
                    if has_ba:
                        nc.tensor.matmul(
                            v_ps[:, 0 : NHL * D], ones128, bav_sb,
                            start=False, stop=True,
                        )
                    nc.vector.tensor_copy(
                        out=vaug4[:, tb, :, 0:64],
                        in_=v_ps[:, 0 : NHL * D].rearrange(
                            "p (h e) -> p h e", e=64
                        ),
                    )

                def k2_shift():
                    # nt3[0:64] = k2 (from nt2's upper half) so h2's scores
                    # get base-aligned operands without a 4th matmul group
                    nc.gpsimd.dma_start(
                        out=qkT[0:64, 3, ts(J, 512)],
                        in_=qkT[64:128, 2, ts(J, 512)],
                    )

                return (
                    [lambda nt=nt: qk_group(nt) for nt in (0, 1)]
                    + [lambda sub=sub: v_group(sub) for sub in range(4)]
                    + [lambda: qk_group(2), k2_shift]
                )

            def qkv_block(J):
                for part in qkv_parts(J):
                    part()

            def koff(Jq, kt):
                # first valid q column for (possibly diagonal) ktile kt
                return 128 * (kt - 4 * Jq) if kt >= 4 * Jq else 0

            def scores_exp_h01(Jq, kt, nkt):
                J = Jq
                off = koff(Jq, kt)
                sc = psA.tile([128, 1024], f32, tag="psa")
                nc.tensor.matmul(
                    sc[:, off:512], k_ap(0, kt), q_ap(0, J)[:, off:512],
                    start=True, stop=True,
                )
                nc.tensor.matmul(
                    sc[:, 512 + off : 1024], k_ap(1, kt),
                    q_ap(1, J)[:, off:512],
                    start=True, stop=True,
                )
                ex = expp.tile([128, 1024], bf16, tag="ex")
                sc2 = sc.rearrange("p (two n) -> p two n", two=2)
                ex2 = ex.rearrange("p (two n) -> p two n", two=2)
                # mid-loop tiles put head 1's exp on DVE (Schraudolph) to
                # unload the ACT bottleneck; boundary tiles stay fully on ACT
                # so DVE's finalize bursts don't stall the PV pipeline.
                h1_dve = SCHRAUD and 2 <= kt < nkt - 3 and kt % 6 != 5
                if h1_dve:
                    nc.scalar.activation(
                        ex2[:, 0, off:512], sc2[:, 0, off:512],
                        AF.Exp, scale=0.125,
                    )
                    dve_exp(ex2[:, 1, off:512], sc2[:, 1, off:512])
                else:
                    nc.scalar.activation(
                        ex2[:, :, off:512], sc2[:, :, off:512],
                        AF.Exp, scale=0.125,
                    )
                if kt >= 4 * J:
                    m4 = maskshh_sb.rearrange(
                        "p (j two n) -> p j two n", j=4, two=2
                    )
                    nc.vector.tensor_mul(
                        ex2[:, :, off:512],
                        ex2[:, :, off:512],
                        m4[:, kt - 4 * J, :, off:512],
                    )
                return ex

            def scores_exp_h2(Jq, g, npair):
                J = Jq
                offa, offb = koff(Jq, 2 * g), koff(Jq, 2 * g + 1)
                sc = psA.tile([128, 1024], f32, tag="psa")
                nc.tensor.matmul(
                    sc[:, offa:512], k_ap(2, 2 * g), q_ap(2, J)[:, offa:512],
                    start=True, stop=True,
                )
                nc.tensor.matmul(
                    sc[:, 512 + offb : 1024], k_ap(2, 2 * g + 1),
                    q_ap(2, J)[:, offb:512],
                    start=True, stop=True,
                )
                ex = expp.tile([128, 1024], bf16, tag="ex")
                b_dve = SCHRAUD and 1 <= g < npair - 2 and g % 3 != 2
                if offa == offb and not b_dve:
                    sc2 = sc.rearrange("p (two n) -> p two n", two=2)
                    ex2 = ex.rearrange("p (two n) -> p two n", two=2)
                    nc.scalar.activation(
                        ex2[:, :, offa:512], sc2[:, :, offa:512],
                        AF.Exp, scale=0.125,
                    )
                else:
                    nc.scalar.activation(
                        ex[:, offa:512], sc[:, offa:512], AF.Exp, scale=0.125
                    )
                    if b_dve:
                        dve_exp(
                            ex[:, 512 + offb : 1024], sc[:, 512 + offb : 1024]
                        )
                    else:
                        nc.scalar.activation(
                            ex[:, 512 + offb : 1024], sc[:, 512 + offb : 1024],
                            AF.Exp, scale=0.125,
                        )
                if g >= 2 * Jq:
                    ja, jb = 2 * g - 4 * Jq, 2 * g + 1 - 4 * Jq
                    m4 = masks_sb.rearrange("p (j n) -> p j n", j=4)
                    nc.vector.tensor_mul(
                        ex[:, offa:512],
                        ex[:, offa:512],
                        m4[:, ja, offa:512],
                    )
                    nc.vector.tensor_mul(
                        ex[:, 512 + offb : 1024],
                        ex[:, 512 + offb : 1024],
                        m4[:, jb, offb:512],
                    )
                return ex

            # ---- pipelined main loop ----
            xTb0 = xTp.tile([128, CT, 512], bf16, tag="xT")
            nc.scalar.dma_start(out=xTb0, in_=xT_r[:, :, ts(0, 512)])
            xT_tiles[0] = xTb0
            nc.gpsimd.dma_start(
                out=wv_sb, in_=wv_d.rearrange("(ct p) n -> p ct n", p=128)
            )
            prefetch_x(1)
            nc.sync.dma_start(out=maskshh_sb, in_=maskshh_d)
            qkv_block(0)
            nc.gpsimd.dma_start(out=ident_sb, in_=ident_d)
            nc.sync.dma_start(out=masks_sb, in_=masks_d)
            nc.gpsimd.dma_start(out=wp01_sb, in_=wp01_d)
            nc.gpsimd.dma_start(out=wp2_sb, in_=wp2_d)
            pending_proj = []
            for J in range(QB):
                nkt = 4 * (J + 1)
                prefetch_x(J + 2)
                psV = psVp.tile([128, 1024], f32, tag="psv")

                def pv_h01(kt, ex):
                    s0 = max(0, kt - 4 * J)
                    for s in range(s0, 4):
                        for h in (0, 1):
                            c = pvcol(s, h)
                            # bank0 group opens at (0,0,0), closes at
                            # (nkt-2,2,1); bank1 opens at (0,3,0), closes in h2
                            st = kt == 0 and ((s == 0 and h == 0) or (s == 3 and h == 0))
                            sp = kt == nkt - 2 and s == 2 and h == 1
                            nc.tensor.matmul(
                                psV[:, c : c + 65],
                                ex[:, 512 * h + 128 * s : 512 * h + 128 * (s + 1)],
                                vaug4[:, kt, h, :],
                                start=st, stop=sp,
                            )

                def pv_h2(g, ex):
                    for kt, base in ((2 * g, 0), (2 * g + 1, 512)):
                        s0 = max(0, kt - 4 * J)
                        for s in range(s0, 4):
                            c = pvcol(s, 2)
                            sp = kt == nkt - 1 and s == 3
                            nc.tensor.matmul(
                                psV[:, c : c + 65],
                                ex[:, base + 128 * s : base + 128 * (s + 1)],
                                vaug4[:, kt, 2, :],
                                start=False, stop=sp,
                            )

                sums = sumsp.tile([128, 16], f32, tag="sums")
                rec = recp.tile([128, 16], f32, tag="rec")
                yts01 = yts01p.tile([128, 4, 128], bf16, tag="yts01")
                yts2 = yts2p.tile([128, 2, 128], bf16, tag="yts2")
                yT01 = yT01p.tile([128, 4, 128], bf16, tag="yT01")
                yT2 = yT2p.tile([128, 2, 128], bf16, tag="yT2")

                def finalize_h01():
                    # gather the 8 h01 sums (6 in bank0, 2 in bank1), one
                    # reciprocal, then two broadcast-normalize evacs
                    psV0 = psV[:, 0:390].rearrange("p (k e) -> p k e", e=65)
                    psV1 = psV[:, 512:642].rearrange("p (k e) -> p k e", e=65)
                    nc.vector.tensor_copy(
                        out=sums[:, 0:6],
                        in_=psV0[:, :, 64:65].rearrange("p k e -> p (k e)"),
                    )
                    nc.vector.tensor_copy(
                        out=sums[:, 6:8],
                        in_=psV1[:, :, 64:65].rearrange("p k e -> p (k e)"),
                    )
                    nc.vector.reciprocal_approx_fast(rec[:, 0:8], sums[:, 0:8])
                    yts01f = yts01.rearrange("p s x -> p (s x)")
                    nc.vector.tensor_tensor(
                        out=yts01f[:, 0:384].rearrange("p (k e) -> p k e", e=64),
                        in0=psV0[:, :, 0:64],
                        in1=rec[:, 0:6].unsqueeze(-1).broadcast_to([128, 6, 64]),
                        op=ALU.mult,
                    )
                    nc.vector.tensor_tensor(
                        out=yts01f[:, 384:512].rearrange(
                            "p (k e) -> p k e", e=64
                        ),
                        in0=psV1[:, :, 0:64],
                        in1=rec[:, 6:8].unsqueeze(-1).broadcast_to([128, 2, 64]),
                        op=ALU.mult,
                    )
                def finalize_h2():
                    psV2 = psV[:, 642:902].rearrange("p (k e) -> p k e", e=65)
                    nc.vector.tensor_copy(
                        out=sums[:, 8:12],
                        in_=psV2[:, :, 64:65].rearrange("p k e -> p (k e)"),
                    )
                    nc.vector.reciprocal_approx_fast(rec[:, 8:12], sums[:, 8:12])
                    nc.vector.tensor_tensor(
                        out=yts2.rearrange("p s x -> p (s x)").rearrange(
                            "p (k e) -> p k e", e=64
                        ),
                        in0=psV2[:, :, 0:64],
                        in1=rec[:, 8:12].unsqueeze(-1).broadcast_to([128, 4, 64]),
                        op=ALU.mult,
                    )
                def transpose_part(src_ap, dst_ap):
                    pt = psA.tile([128, 1024], f32, tag="psa")
                    ptb = pt[:, 0:64].bitcast(bf16)
                    nc.tensor.transpose(ptb, src_ap, ident_sb)
                    nc.vector.tensor_copy(out=dst_ap, in_=ptb)

                # --- h01 phase, pipelined: scores(kt+1) ahead of PV(kt),
                # proj(J-1) parts drained proportionally ---
                pp = pending_proj
                pending_proj = []
                ppi = 0
                exq = [scores_exp_h01(J, 0, nkt)]
                if nkt > 1:
                    exq.append(scores_exp_h01(J, 1, nkt))
                for kt in range(nkt):
                    if kt + 2 < nkt:
                        exq.append(scores_exp_h01(J, kt + 2, nkt))
                    pv_h01(kt, exq.pop(0))
                    want = (kt + 1) * len(pp) // nkt
                    while ppi < want:
                        pp[ppi]()
                        ppi += 1
                finalize_h01()
                while ppi < len(pp):
                    pp[ppi]()
                    ppi += 1
                qparts = qkv_parts(J + 1) if J + 1 < QB else []

                # --- h2 phase: ktile pairs, qkv(J+1) parts drained ---
                qpi = 0
                npair = nkt // 2
                exq = [scores_exp_h2(J, 0, npair)]
                if npair > 1:
                    exq.append(scores_exp_h2(J, 1, npair))
                for g in range(npair):
                    if g + 2 < npair:
                        exq.append(scores_exp_h2(J, g + 2, npair))
                    pv_h2(g, exq.pop(0))
                    want = (g + 1) * len(qparts) // npair
                    while qpi < want:
                        qparts[qpi]()
                        qpi += 1
                finalize_h2()
                while qpi < len(qparts):
                    qparts[qpi]()
                    qpi += 1

                # --- projection closures for this J (run during J+1) ---
                def proj_parts(J, yT01, yT2):
                    def jj_part(jj):
                        p, half = jj // 2, jj % 2
                        y2 = yT2[64 * half : 64 * half + 64, p, :]
                        w2 = wp2_sb[64 * half : 64 * half + 64, :]
                        pt = psA.tile([128, 1024], f32, tag="psa")
                        pj, pj2 = pt[:, 0:512], pt[:, 512:768]
                        nc.tensor.matmul(
                            pj, yT01[:, jj, :], wp01_sb[:, 0:512],
                            start=True, stop=False,
                        )
                        nc.tensor.matmul(
                            pj, y2, w2[:, 0:512], start=False, stop=True
                        )
                        nc.tensor.matmul(
                            pj2, yT01[:, jj, :], wp01_sb[:, 512:768],
                            start=True, stop=False,
                        )
                        nc.tensor.matmul(
                            pj2, y2, w2[:, 512:768],
                            start=False, stop=True,
                        )
                        acc = accp.tile([128, C], bf16, tag="acc")
                        nc.vector.tensor_copy(out=acc[:, 0:512], in_=pj)
                        nc.vector.tensor_copy(out=acc[:, 512:768], in_=pj2)
                        nc.sync.dma_start(
                            out=out_d[ds(J * 512 + jj * 128, 128), :], in_=acc
                        )

                    return [lambda jj=jj: jj_part(jj) for jj in range(4)]

                tparts = [
                    (lambda a=yts01, b=yT01, s=s: transpose_part(
                        a[:, s, :], b[:, s, :]
                    ))
                    for s in range(4)
                ] + [
                    (lambda a=yts2, b=yT2, p=p: transpose_part(
                        a[:, p, :], b[:, p, :]
                    ))
                    for p in range(2)
                ]
                pending_proj = tparts + proj_parts(J, yT01, yT2)
            for part in pending_proj:
                part()
    nc.compile()
    return nc


def _host_masks():
    import ml_dtypes

    p = np.arange(128)[:, None]
    f = np.arange(512)[None, :]
    m = np.zeros((128, 4, 512), np.float32)
    for j in range(4):
        m[:, j, :] = (f >= p + 128 * j).astype(np.float32)
    pairs = m.reshape(128, 2048).astype(ml_dtypes.bfloat16)
    dup = np.concatenate([m, m], axis=2).reshape(128, 4096)  # [m_j | m_j] per j
    return pairs, dup.astype(ml_dtypes.bfloat16)


def _core_inputs(xT, w_attn, b_attn, hg, t_len, has_ba):
    # column ranges of this head-group inside w_attn: q | k | v blocks of C each
    q0 = 192 * hg
    wqk = np.zeros((C, 3, 128), np.float32)
    wqk[:, 0, 0:64] = w_attn[:, q0 : q0 + 64]                 # q0
    wqk[:, 0, 64:128] = w_attn[:, q0 + 64 : q0 + 128]         # q1
    wqk[:, 1, 0:64] = w_attn[:, C + q0 : C + q0 + 64]         # k0
    wqk[:, 1, 64:128] = w_attn[:, C + q0 + 64 : C + q0 + 128] # k1
    wqk[:, 2, 0:64] = w_attn[:, q0 + 128 : q0 + 192]          # q2
    wqk[:, 2, 64:128] = w_attn[:, C + q0 + 128 : C + q0 + 192]  # k2
    mp, mh = _host_masks()
    import ml_dtypes
    ident = np.eye(128, dtype=np.float32).astype(ml_dtypes.bfloat16)
    ins = {
        "xT": xT,
        "ident": ident,
        "wqk": np.ascontiguousarray(wqk.reshape(C, 384)).astype(
            ml_dtypes.bfloat16
        ),
        "wv": np.ascontiguousarray(
            w_attn[:, 2 * C + q0 : 2 * C + q0 + 192]
        ).astype(ml_dtypes.bfloat16),
        "masks": mp,
        "maskshh": mh,
    }
    if has_ba:
        baqk = np.zeros((1, 384), np.float32)
        baqk[0, 0:64] = b_attn[q0 : q0 + 64]
        baqk[0, 64:128] = b_attn[q0 + 64 : q0 + 128]
        baqk[0, 128:192] = b_attn[C + q0 : C + q0 + 64]
        baqk[0, 192:256] = b_attn[C + q0 + 64 : C + q0 + 128]
        baqk[0, 256:320] = b_attn[q0 + 128 : q0 + 192]
        baqk[0, 320:384] = b_attn[C + q0 + 128 : C + q0 + 192]
        ins["baqk"] = baqk
        ins["bav"] = np.ascontiguousarray(
            b_attn[2 * C + q0 : 2 * C + q0 + 192].reshape(1, 192)
        )
    return ins


def kernel(**inputs):
    import ml_dtypes
    from concourse.bass_utils import run_bass_kernel_spmd

    x = np.asarray(inputs["x"], dtype=np.float32)
    w_attn = np.asarray(inputs["w_attn"], dtype=np.float32)
    b_attn = np.asarray(inputs["b_attn"], dtype=np.float32)
    w_proj = np.asarray(inputs["w_proj"], dtype=np.float32)
    b_proj = np.asarray(inputs["b_proj"], dtype=np.float32)

    has_ba = bool(np.any(b_attn))
    key = (T, has_ba)
    if key not in _PROG_CACHE:
        _PROG_CACHE[key] = _build(T, has_ba)
    nc = _PROG_CACHE[key]

    xTs = [
        np.ascontiguousarray(x[b].T).astype(ml_dtypes.bfloat16)
        for b in range(B)
    ]
    in_maps = []
    for c in range(8):
        b, hg = c // 4, c % 4
        ins = _core_inputs(xTs[b], w_attn, b_attn, hg, T, has_ba)
        wp = w_proj[192 * hg : 192 * hg + 192, :]
        ins["wp01"] = np.ascontiguousarray(wp[0:128, :]).astype(
            ml_dtypes.bfloat16
        )
        ins["wp2"] = np.ascontiguousarray(
            np.concatenate([wp[128:192, :], wp[128:192, :]], axis=0)
        ).astype(ml_dtypes.bfloat16)
        in_maps.append(ins)

    res = run_bass_kernel_spmd(nc, in_maps, core_ids=list(range(8)), trace=TRACE)
    global LAST
    LAST = res
    out = np.zeros((B, T, C), np.float32)
    for c in range(8):
        out[c // 4] += res.results[c]["out"]
    out += b_proj
    return out
